# revision 26
# baseline (speedup 1.0000x reference)
"""Multi-headed causal attention (B=2, S=2048, D=1024, H=16, DK=DV=64) on 8
Trainium2 NeuronCores.

Sharding (zero-communication): cores split into 2 groups of 4, one group per
batch element. Within a group, core g owns four 128-query stripes, one per
"slot" s=0..3 with a key-block budget of 4(s+1) 128-key blocks. Stripe
assignment per group is chosen so every stripe's causal key-need fits its
slot budget:
    g0: stripes [0, 7, 8, 15], g1: [1, 6, 9, 14],
    g2: [2, 5, 10, 13],        g3: [3, 4, 11, 12]
The program is identical on all cores (SPMD); per-core variation lives only
in the data (query permutation in xq_t, 0/1 mask tiles, output row
unpermutation on host).

All matmul operands are bf16; accumulation is fp32 in PSUM. Scores for
key-block kb are computed only for query slots s >= kb//4 ("staircase").
Attention matmuls are padded to the full 128x128 PE array (zero-padded
per-head qT for K=128 scores; M=128 AV stationary whose extra output rows
are ignored) so the PE activity monitor keeps the clock at 2.4 GHz.
exp(0.125*x) runs on the ACT engine per PSUM bank into bf16 am tiles;
masking is a multiplicative 0/1 bf16 op on am touching only the staircase
edge. The softmax denominator comes from an all-ones 65th column in each V
tile; per-head normalization uses DVE reciprocal + PE row-replication on
the small [64, 512] attention output, letting the output projection
accumulate all 16 heads in PSUM.

The whole kernel is one software-pipelined instruction stream: attention
scores+exp for head h interleave into the kT projection as soon as
kT[h//2] exists (filling kT's PSUM-drain stalls and feeding ACT early),
V-projection kb-groups interleave with the score/AV stream, and AV +
normalization drain as V chunks land. PSUM pools are stacked so the
score pool coexists with the kT pool (4+4 banks), then with AV/rep pools
(4+3+1).
"""

import numpy as np

B, S, D, H, DK = 2, 2048, 1024, 16, 64
NQ = 512          # queries per core: 4 slots x 128
NCORES = 8

SLOT_STRIPES = [
    [0, 7, 8, 15],
    [1, 6, 9, 14],
    [2, 5, 10, 13],
    [3, 4, 11, 12],
]

_BUILT = {}


def _build_nc():
    import concourse.bacc as bacc
    import concourse.mybir as mybir
    from concourse import tile

    f32 = mybir.dt.float32
    f32r = mybir.dt.float32r
    bf16 = mybir.dt.bfloat16
    AF = mybir.ActivationFunctionType
    ALU = mybir.AluOpType

    nc = bacc.Bacc("TRN2", target_bir_lowering=False, debug=False,
                   num_devices=NCORES)

    xk_t = nc.declare_dram_parameter("xk_t", [D, S], bf16, isOutput=False)
    xv_t = nc.declare_dram_parameter("xv_t", [D, S], bf16, isOutput=False)
    xq_t = nc.declare_dram_parameter("xq_t", [D, NQ], bf16, isOutput=False)
    wk_t = nc.declare_dram_parameter("wk_t", [D, D], bf16, isOutput=False)
    wv_t = nc.declare_dram_parameter("wv_t", [D, D], bf16, isOutput=False)
    wq_t = nc.declare_dram_parameter("wq_t", [D, D], bf16, isOutput=False)
    wo_t = nc.declare_dram_parameter("wo_t", [D, D], bf16, isOutput=False)
    bk_s = nc.declare_dram_parameter("bk_s", [128, 8], f32, isOutput=False)
    bq_s = nc.declare_dram_parameter("bq_s", [128, 8], f32, isOutput=False)
    bv_rep_d = nc.declare_dram_parameter("bv_rep", [128, D], bf16,
                                         isOutput=False)
    bo_rep_d = nc.declare_dram_parameter("bo_rep", [128, D], bf16,
                                         isOutput=False)
    onesf = nc.declare_dram_parameter("onesf", [128, 128], f32r,
                                      isOutput=False)
    maskin = nc.declare_dram_parameter("maskin", [128, 4 * NQ], bf16,
                                       isOutput=False)
    out = nc.declare_dram_parameter("out", [NQ, D], f32, isOutput=True)

    from contextlib import ExitStack

    AM_CAP = 8    # am pool bufs = max score-units in flight ahead of AV

    with tile.TileContext(nc) as tc:
        with ExitStack() as ctx:
            persist = ctx.enter_context(tc.tile_pool(name="persist", bufs=1))

            # ---- persistent tiles ----
            bk_sb = persist.tile([128, 8], f32, name="bk", tag="bk")
            bq_sb = persist.tile([128, 8], f32, name="bq", tag="bq")
            onesf_sb = persist.tile([128, 128], f32r, name="onesf",
                                    tag="onesf")
            mask_sb = persist.tile([128, 4 * NQ], bf16, name="mask",
                                   tag="mask")
            bv_rep = persist.tile([128, D], bf16, name="bvrep", tag="bvrep")
            bo_rep = persist.tile([128, D], bf16, name="borep", tag="borep")
            kT = [persist.tile([128, S], bf16, name=f"kt{ft}", tag=f"kt{ft}")
                  for ft in range(8)]
            qTz = [persist.tile([128, NQ], bf16, name=f"qt{h}",
                                tag=f"qt{h}")
                   for h in range(H)]
            v = [persist.tile([128, 66 * H + 64], bf16, name=f"v{kb}",
                              tag=f"v{kb}")
                 for kb in range(16)]
            navTn = [persist.tile([128, NQ], bf16, name=f"nv{i}",
                                  tag=f"nv{i}")
                     for i in range(8)]

            # SBUF pool stack (LIFO): wvp/xvp/amp/dnmp outlive p1/p2
            wvp = ctx.enter_context(tc.tile_pool(name="wvp", bufs=1))
            xvp = ctx.enter_context(tc.tile_pool(name="xvp", bufs=1))
            amp = ctx.enter_context(tc.tile_pool(name="amp", bufs=AM_CAP))
            dnmp = ctx.enter_context(tc.tile_pool(name="dnmp", bufs=2))
            p1pool = tc.tile_pool(name="p1pool", bufs=1)
            p1s = p1pool.__enter__()
            p2pool = tc.tile_pool(name="p2pool", bufs=1)
            p2s = p2pool.__enter__()

            xq_sb = [p2s.tile([128, NQ], bf16, name=f"xq{kp}", tag=f"xq{kp}")
                     for kp in range(8)]
            for kp in range(8):
                nc.sync.dma_start(xq_sb[kp][:],
                                  xq_t[kp * 128:(kp + 1) * 128, :])
            nc.sync.dma_start(bq_sb[:], bq_s[:])
            nc.sync.dma_start(bk_sb[:], bk_s[:])
            nc.sync.dma_start(onesf_sb[:], onesf[:])
            nc.sync.dma_start(mask_sb[:], maskin[:])
            nc.sync.dma_start(bv_rep[:], bv_rep_d[:])
            nc.sync.dma_start(bo_rep[:], bo_rep_d[:])

            wk_sb = [p1s.tile([128, D], bf16, name=f"wk{kp}", tag=f"wk{kp}")
                     for kp in range(8)]
            xk_sb = [p1s.tile([128, S], bf16, name=f"xk{kp}", tag=f"xk{kp}")
                     for kp in range(8)]
            for kp in range(8):
                nc.sync.dma_start(wk_sb[kp][:],
                                  wk_t[kp * 128:(kp + 1) * 128, :])
            for kp in range(8):
                nc.sync.dma_start(xk_sb[kp][:],
                                  xk_t[kp * 128:(kp + 1) * 128, :])
            wv_sb = [wvp.tile([128, D], bf16, name=f"wv{kp}", tag=f"wv{kp}")
                     for kp in range(8)]
            for kp in range(8):
                nc.sync.dma_start(wv_sb[kp][:],
                                  wv_t[kp * 128:(kp + 1) * 128, :])

            for h in range(H):
                nc.gpsimd.memset(qTz[h][:], 0.0)
            for kb in range(16):
                nc.gpsimd.memset(
                    v[kb][:, 0:66 * H].rearrange("p (h c) -> p h c",
                                                 c=66)[:, :, 64:65],
                    1.0)
                nc.gpsimd.memset(v[kb][:, 66 * H:], 0.0)

            # ---- qT projection ----
            with tc.tile_pool(name="psq", bufs=2, space="PSUM") as psq, \
                 tc.tile_pool(name="wqp", bufs=2) as wqp:
                for ft in range(8):
                    wqc = [wqp.tile([128, 128], bf16, name=f"wqc{kp}",
                                    tag=f"wqc{kp}")
                           for kp in range(8)]
                    for kp in range(8):
                        nc.sync.dma_start(
                            wqc[kp][:],
                            wq_t[kp * 128:(kp + 1) * 128,
                                 ft * 128:(ft + 1) * 128])
                    ps = psq.tile([128, NQ], f32, name="pq", tag="pq")
                    for kp in range(8):
                        nc.tensor.matmul(
                            ps[:], wqc[kp][:],
                            xq_sb[kp][:], start=(kp == 0), stop=(kp == 7))
                    for i in range(2):
                        nc.vector.tensor_scalar_add(
                            qTz[2 * ft + i][64 * i:64 * i + 64, :],
                            ps[64 * i:64 * i + 64, :],
                            bq_sb[64 * i:64 * i + 64, ft:ft + 1])
            p2pool.__exit__(None, None, None)

            # ---- pipelined kT proj + V proj + attention ----
            scp = tc.tile_pool(name="scp", bufs=3, space="PSUM")
            scps = scp.__enter__()

            import collections
            units = [(h, pr) for h in range(H) for pr in range(8)]
            state = {
                "si": 0,          # next unit to emit scores for
                "am_out": 0,      # am tiles allocated minus consumed
                "ft_done": -1,
                "vg_done": -1,
                "emits": 0,
                "avpp": None, "repp": None,
            }
            av_q = collections.deque()
            avt = {}
            amt = {}
            dnms = {}
            deferred = []

            def emit_scores():
                h, pr = units[state["si"]]
                state["si"] += 1
                state["am_out"] += 1
                state["emits"] += 1
                hp = h // 2
                q = pr // 2
                Nq = 512 - 128 * q
                qoff = 128 * q
                am = amp.tile([128, 1024], bf16, name="am", tag="am")
                for u in range(2):
                    kb = 2 * pr + u
                    sc = scps.tile([128, 512], f32, name="sc", tag="sc")
                    nc.tensor.matmul(
                        sc[:, 0:Nq],
                        kT[hp][:, kb * 128:(kb + 1) * 128],
                        qTz[h][:, qoff:512],
                        start=True, stop=True)
                    nc.scalar.activation(
                        am[:, 512 * u:512 * u + Nq], sc[:, 0:Nq],
                        AF.Exp, scale=0.125)
                amv = am[:].rearrange("p (u c) -> p u c", c=512)[:, :, 0:128]
                moff = 256 * pr
                mkv = mask_sb[:, moff:moff + 256].rearrange(
                    "p (u c) -> p u c", c=128)
                eng = nc.vector if h % 2 == 0 else nc.gpsimd
                eng.tensor_tensor(amv, amv, mkv, ALU.mult)
                amt[(h, pr)] = am
                av_q.append((h, pr))

            def make_norm(hp):
                def fn():
                    rcp = dnmp.tile([128, NQ], f32r, name="rcp",
                                    tag="rcp", bufs=1)
                    with nc.allow_low_precision(reason="f32r recip"):
                        nc.vector.reciprocal(rcp[:], dnms.pop(hp)[:])
                    for i in range(2):
                        rep = state["repp"].tile([128, NQ], f32, name="rep",
                                                 tag="rep")
                        nc.tensor.matmul(
                            rep[0:64, :],
                            onesf_sb[64 * i:64 * i + 1, 0:64],
                            rcp[64 * i:64 * i + 1, :],
                            start=True, stop=True)
                        repS = dnmp.tile([128, NQ], f32, name="repS",
                                         tag="repS", bufs=1)
                        nc.vector.tensor_copy(repS[0:64, :], rep[0:64, :])
                        with nc.allow_low_precision(reason="softmax norm"):
                            nc.vector.tensor_tensor(
                                navTn[hp][64 * i:64 * i + 64, :],
                                avt.pop(2 * hp + i)[0:64, :],
                                repS[0:64, :], ALU.mult)
                return fn

            def emit_av():
                h, pr = av_q.popleft()
                state["am_out"] -= 1
                state["emits"] += 1
                if pr == 0:
                    avt[h] = state["avpp"].tile([128, NQ], f32, name="av",
                                                tag="av")
                q = pr // 2
                Nq = 512 - 128 * q
                qoff = 128 * q
                am = amt.pop((h, pr))
                for u in range(2):
                    kb = 2 * pr + u
                    nc.tensor.matmul(
                        avt[h][:, qoff:qoff + Nq],
                        v[kb][:, 66 * h:66 * h + 128],
                        am[:, 512 * u:512 * u + Nq],
                        start=(kb == 0), stop=(kb == 15))
                if pr == 7:
                    # denominator copy now; recip/replicate/mult deferred
                    hp = h // 2
                    if h % 2 == 0:
                        dnms[hp] = dnmp.tile([128, NQ], f32, name="dnm",
                                             tag="dnm")
                    nc.scalar.copy(
                        dnms[hp][64 * (h % 2):64 * (h % 2) + 1, :],
                        avt[h][64:65, :])
                    if h % 2 == 1:
                        deferred.append([state["emits"] + 2,
                                         make_norm(hp)])

            def run_due():
                still = []
                for item in deferred:
                    if state["emits"] >= item[0]:
                        item[1]()
                    else:
                        still.append(item)
                deferred[:] = still

            def scores_ready():
                if state["si"] >= len(units) or state["am_out"] >= AM_CAP:
                    return False
                h, pr = units[state["si"]]
                return h // 2 <= state["ft_done"]

            def av_ready():
                if not av_q or state["avpp"] is None:
                    return False
                h, pr = av_q[0]
                if 2 * pr + 1 > state["vg_done"]:
                    return False
                return h in avt or len(avt) < 2

            def pump():
                while True:
                    run_due()
                    if av_ready():
                        emit_av()
                        continue
                    if scores_ready():
                        emit_scores()
                        continue
                    break

            # kT projection with interleaved early score units
            with tc.tile_pool(name="psk", bufs=1, space="PSUM") as psk:
                for ft in range(8):
                    ps4 = [psk.tile([128, 512], f32, name=f"pk{sc}",
                                    tag=f"pk{sc}")
                           for sc in range(4)]
                    for kp in range(8):
                        for sc in range(4):
                            nc.tensor.matmul(
                                ps4[sc][:],
                                wk_sb[kp][:, ft * 128:(ft + 1) * 128],
                                xk_sb[kp][:, 512 * sc:512 * (sc + 1)],
                                start=(kp == 0), stop=(kp == 7))
                    for sc in range(4):
                        nc.vector.tensor_scalar_add(
                            kT[ft][:, 512 * sc:512 * (sc + 1)],
                            ps4[sc][:], bk_sb[:, ft:ft + 1])
                    state["ft_done"] = ft
                    # fill the psk drain stall with two score units
                    for _ in range(2):
                        if scores_ready():
                            emit_scores()
            p1pool.__exit__(None, None, None)

            avpp = tc.tile_pool(name="avp", bufs=2, space="PSUM")
            state["avpp"] = avpp.__enter__()
            repp = tc.tile_pool(name="repp", bufs=1, space="PSUM")
            state["repp"] = repp.__enter__()
            wop = tc.tile_pool(name="wop", bufs=1)
            wops = wop.__enter__()
            fop = tc.tile_pool(name="fop", bufs=2)
            fops = fop.__enter__()
            wo_sb = [wops.tile([128, D], bf16, name=f"wo{hp}", tag=f"wo{hp}")
                     for hp in range(8)]
            for hp in range(8):
                nc.sync.dma_start(wo_sb[hp][:],
                                  wo_t[hp * 128:(hp + 1) * 128, :])

            # V projection interleaved with the attention stream
            with tc.tile_pool(name="psv", bufs=1, space="PSUM") as psv:
                for chunk in range(4):
                    xvc = [xvp.tile([128, 512], bf16, name=f"xv{kp}",
                                    tag=f"xv{kp}")
                           for kp in range(8)]
                    for kp in range(8):
                        nc.sync.dma_start(
                            xvc[kp][:],
                            xv_t[kp * 128:(kp + 1) * 128,
                                 chunk * 512:(chunk + 1) * 512])
                    for stl in range(4):
                        kb = 4 * chunk + stl
                        pv = [psv.tile([128, 512], f32, name=f"pv{half}",
                                       tag=f"pv{half}")
                              for half in range(2)]
                        for kp in range(8):
                            for half in range(2):
                                nc.tensor.matmul(
                                    pv[half][:],
                                    xvc[kp][:, stl * 128:(stl + 1) * 128],
                                    wv_sb[kp][:, 512 * half:
                                              512 * (half + 1)],
                                    start=(kp == 0), stop=(kp == 7))
                        for half in range(2):
                            nc.vector.tensor_tensor(
                                v[kb][:, 528 * half:528 * (half + 1)]
                                .rearrange("p (h c) -> p h c", c=66)[
                                    :, :, 0:64],
                                pv[half][:].rearrange("p (h c) -> p h c",
                                                      c=64),
                                bv_rep[:, 512 * half:512 * (half + 1)]
                                .rearrange("p (h c) -> p h c", c=64),
                                ALU.add)
                        state["vg_done"] = kb
                        pump()

            # drain the rest of the attention stream
            guard = 0
            while av_q or state["si"] < len(units) or deferred:
                pump()
                if deferred and not av_ready() and not scores_ready():
                    # force the earliest deferred norm to free an avp slot
                    item = deferred.pop(0)
                    item[1]()
                guard += 1
                assert guard < 10000, "pipeline deadlock"
            repp.__exit__(None, None, None)
            avpp.__exit__(None, None, None)

            # ---- output projection, all heads PSUM-accumulated ----
            with tc.tile_pool(name="pso", bufs=2, space="PSUM") as pso:
                for rc in range(4):
                    po = [pso.tile([128, 512], f32, name=f"po{oc}",
                                   tag=f"po{oc}")
                          for oc in range(2)]
                    for hp in range(8):
                        for oc in range(2):
                            nc.tensor.matmul(
                                po[oc][:],
                                navTn[hp][:, rc * 128:(rc + 1) * 128],
                                wo_sb[hp][:, 512 * oc:512 * (oc + 1)],
                                start=(hp == 0), stop=(hp == 7))
                    fo = fops.tile([128, 1024], f32, name="fo", tag="fo")
                    for oc in range(2):
                        nc.vector.tensor_tensor(
                            fo[:, 512 * oc:512 * (oc + 1)],
                            po[oc][:],
                            bo_rep[:, 512 * oc:512 * (oc + 1)], ALU.add)
                    nc.sync.dma_start(out[rc * 128:(rc + 1) * 128, :],
                                      fo[:])
            fop.__exit__(None, None, None)
            wop.__exit__(None, None, None)
            scp.__exit__(None, None, None)
    nc.compile()
    return nc


def kernel(V, K, Q, padding_mask, Wv_w, Wv_b, Wk_w, Wk_b, Wq_w, Wq_b,
           Wo_w, Wo_b):
    from concourse.bass_utils import run_bass_kernel_spmd
    import ml_dtypes

    bf = ml_dtypes.bfloat16
    V = np.asarray(V, np.float32)
    K = np.asarray(K, np.float32)
    Q = np.asarray(Q, np.float32)
    padding_mask = np.asarray(padding_mask)

    if "nc" not in _BUILT:
        _BUILT["nc"] = _build_nc()
    nc = _BUILT["nc"]

    wk_t = np.ascontiguousarray(np.asarray(Wk_w, np.float32).T).astype(bf)
    wv_t = np.ascontiguousarray(np.asarray(Wv_w, np.float32).T).astype(bf)
    wq_t = np.ascontiguousarray(np.asarray(Wq_w, np.float32).T).astype(bf)
    wo_t = np.ascontiguousarray(np.asarray(Wo_w, np.float32).T).astype(bf)
    bk_s = np.ascontiguousarray(
        np.asarray(Wk_b, np.float32).reshape(8, 128).T)
    bq_s = np.ascontiguousarray(
        np.asarray(Wq_b, np.float32).reshape(8, 128).T)
    bv_rep = np.ascontiguousarray(np.broadcast_to(
        np.asarray(Wv_b, np.float32).reshape(1, D), (128, D))).astype(bf)
    bo_rep = np.ascontiguousarray(np.broadcast_to(
        np.asarray(Wo_b, np.float32).reshape(1, D), (128, D))).astype(bf)
    onesf = np.ones((128, 128), np.float32)

    xk_T = [np.ascontiguousarray(K[b].T).astype(bf) for b in range(B)]
    xv_T = [np.ascontiguousarray(V[b].T).astype(bf) for b in range(B)]

    in_maps = []
    blocks = []
    for core in range(NCORES):
        b, g = core // 4, core % 4
        stripes = SLOT_STRIPES[g]
        blocks.append((b, stripes))
        rows = np.concatenate(
            [np.arange(128 * st, 128 * st + 128) for st in stripes])
        xq_t = np.ascontiguousarray(Q[b][rows].T).astype(bf)
        maskin = np.zeros((128, 4 * NQ), np.float32)
        karange = np.arange(128)
        for q in range(4):
            qidx = 128 * stripes[q] + karange
            for t in range(4):
                kb = 4 * q + t
                kidx = 128 * kb + karange
                valid = (kidx[:, None] <= qidx[None, :]) & \
                        (padding_mask[b][kidx][:, None] != 0)
                maskin[:, 512 * q + 128 * t:512 * q + 128 * t + 128] = \
                    np.where(valid, 1.0, 0.0)
        in_maps.append({
            "xk_t": xk_T[b], "xv_t": xv_T[b], "xq_t": xq_t,
            "wk_t": wk_t, "wv_t": wv_t, "wq_t": wq_t, "wo_t": wo_t,
            "bk_s": bk_s, "bq_s": bq_s,
            "bv_rep": bv_rep, "bo_rep": bo_rep,
            "onesf": onesf,
            "maskin": maskin.astype(bf),
        })

    _BUILT["last_maps"] = in_maps
    res = run_bass_kernel_spmd(nc, in_maps, core_ids=list(range(NCORES)))
    _BUILT["last_result"] = res

    outf = np.empty((B, S, D), np.float32)
    for core in range(NCORES):
        b, stripes = blocks[core]
        o = res.results[core]["out"]
        for s, st in enumerate(stripes):
            outf[b, 128 * st:128 * st + 128] = o[128 * s:128 * s + 128]
    return outf
